# revision 54
# baseline (speedup 1.0000x reference)
"""MoE layer (14 routed top-2 + 2 shared experts) on 8 trn2 NeuronCores.

Strategy: data-parallel over tokens. Each core takes 1024 of the 8192
tokens and runs the complete MoE for them: router matmul -> softmax ->
top-2 gate extraction, then all 16 expert FFNs computed densely with the
gate (zero for non-selected experts) folded into the accumulation. No
collectives needed; the host concatenates the 8 token shards.

Layouts (all chosen so no on-device transpose is ever needed):
  xT   [D, tok]   feature-major   -> mm1 moving operand + router lhsT
  h    [I, tok]   feature-major   -> mm1 output, mm2 stationary operand
  y    [tok, D]   token-major     -> mm2 output; gate is a per-partition
                                     scalar there, so gating+accumulate is
                                     one fused DVE scalar_tensor_tensor.
Matmuls run in float32r (full PE rate at moving-dim >= 256; fp32 bits).

Walrus codegen only accepts ONE sync-wait on most ISA structs (LDWEIGHTS,
TensorReduce, DMA, NoOp, Branch...), but Tile's scheduler freely emits
several per instruction. _legalize_bir() post-processes the serialized
BIR: same-semaphore waits are merged (max value) and every excess wait is
hoisted onto an injected single-wait NoOp right before the instruction on
the same engine queue — semantically identical, structurally legal.
"""

import os
import sys

sys.path.insert(0, "/opt/trn_rl_repo")
# recover cleanly if a previous run left the NeuronCores wedged
os.environ.setdefault("NEURON_RT_RESET_CORES", "1")

from contextlib import ExitStack

import numpy as np
import orjson

import concourse.bass as bass
import concourse.tile as tile
from concourse import mybir
from concourse.bass_utils import run_bass_kernel_spmd

P = 128
B, T, D = 4, 2048, 1024
I = 1024
E_ROUTED = 14
N_SHARED = 2
E = E_ROUTED + N_SHARED  # 16 FFNs total; 0..13 routed (gated), 14..15 shared
NCORES = 8
NTOK = B * T  # 8192
TOK = NTOK // NCORES  # 1024 tokens per core
T_TILES = TOK // P  # 8
D_TILES = D // P  # 8
I_TILES = I // P  # 8
FD = 512  # matmul moving free dim (fp32 max)
NCH_TOK = TOK // FD  # 2
NCH_D = D // FD  # 2

f32 = mybir.dt.float32
f32r = mybir.dt.float32r
AX = mybir.AxisListType.X
OP = mybir.AluOpType
ACT = mybir.ActivationFunctionType


def _legalize_bir(js_bytes: bytes) -> bytes:
    """Merge same-sem waits and hoist excess waits to injected NoOps so no
    instruction carries more than one sync-wait (walrus struct limits)."""
    js = orjson.loads(js_bytes)
    n = 0
    for fn in js["functions"]:
        for blk in fn["blocks"]:
            new_insts = []
            for inst in blk["instructions"]:
                si = inst.get("sync_info")
                waits = (si or {}).get("on_wait") or []
                if len(waits) > 1:
                    merged, order, other = {}, [], []
                    for w in waits:
                        if (
                            w.get("wait_mode") == "sem-ge-imm"
                            and w.get("sync_type") == "semaphore"
                            and w.get("wait_reg") is None
                        ):
                            k = w["id"]
                            if k in merged:
                                if w["wait_value"] > merged[k]["wait_value"]:
                                    merged[k] = w
                            else:
                                merged[k] = w
                                order.append(k)
                        else:
                            other.append(w)
                    ws = [merged[k] for k in order] + other
                    for w in ws[:-1]:
                        n += 1
                        new_insts.append(
                            {
                                "debug": inst.get("debug", 0),
                                "engine": inst["engine"],
                                "name": f"I-lgl-{n}",
                                "opcode": "NoOp",
                                "sync_info": {"on_update": [], "on_wait": [w]},
                            }
                        )
                    si["on_wait"] = ws[-1:]
                new_insts.append(inst)
            blk["instructions"] = new_insts
    return orjson.dumps(js)


def _build_program(has_rb: bool, has_b1: bool, has_b2: bool) -> bass.Bass:
    nc = bass.Bass()

    xT_d = nc.dram_tensor("xT", [D, TOK], f32r, kind="ExternalInput")
    x_d = nc.dram_tensor("x", [TOK, D], f32, kind="ExternalInput")
    # router inputs as 3-way bf16 splits (x = xh+xm+xl exactly, same for
    # rw): 6 bf16 product groups accumulated in fp32 PSUM give true-fp32
    # logits; the PE's native fp32 mode is ~1e-5 off, enough to flip top-2
    # selections on tokens whose #2/#3 affinity gap is ~4e-6.
    bf16 = mybir.dt.bfloat16
    xs_d = {
        p: nc.dram_tensor(f"xs_{p}", [D, TOK], bf16, kind="ExternalInput")
        for p in "hml"
    }
    rs_d = {
        p: nc.dram_tensor(f"rs_{p}", [D, E_ROUTED], bf16, kind="ExternalInput")
        for p in "hml"
    }
    rb_d = nc.dram_tensor("rb", [1, E_ROUTED], f32, kind="ExternalInput")
    w1_d = nc.dram_tensor("w1", [E, D_TILES, P, I], f32r, kind="ExternalInput")
    b1_d = nc.dram_tensor("b1", [P, I_TILES * E], f32, kind="ExternalInput")
    w2_d = nc.dram_tensor("w2", [E, I_TILES, P, D], f32r, kind="ExternalInput")
    b2_d = nc.dram_tensor("b2", [E, D], f32r, kind="ExternalInput")
    out_d = nc.dram_tensor("out", [TOK, D], f32, kind="ExternalOutput")

    with tile.TileContext(nc) as tc, ExitStack() as ctx:
        const = ctx.enter_context(tc.tile_pool(name="const", bufs=1))
        rtmp = ctx.enter_context(tc.tile_pool(name="rtmp", bufs=T_TILES))
        ps1 = ctx.enter_context(tc.tile_pool(name="ps1", bufs=4, space="PSUM"))
        ps2 = ctx.enter_context(tc.tile_pool(name="ps2", bufs=4, space="PSUM"))

        # ---- resident tensors -------------------------------------------
        xT_sb = const.tile([P, D_TILES * TOK], f32r, name="xT_sb")
        xs_sb = const.tile([P, D_TILES * TOK], mybir.dt.bfloat16, name="xs_sb")
        rs_sb = {}
        for p in "hml":
            t = const.tile(
                [P, D_TILES * E_ROUTED], mybir.dt.bfloat16, name=f"rs_{p}"
            )
            for k in range(D_TILES):
                nc.sync.dma_start(
                    t[:, k * E_ROUTED : (k + 1) * E_ROUTED],
                    rs_d[p][k * P : (k + 1) * P, :],
                )
            rs_sb[p] = t
        acc = [const.tile([P, D], f32, name=f"acc{m}") for m in range(T_TILES)]
        h_sb = const.tile([P, I_TILES * TOK], f32r, name="h_sb")
        # Persistent weight tiles, allocated once: per-expert overwrites are
        # tracked at range level (pool slot realloc would pull every previous
        # accessor into the dep set).
        w1_sb = const.tile([P, D_TILES * I], f32r, name="w1_sb")
        w2_sb = const.tile([P, I_TILES * D], f32r, name="w2_sb")

        ones_sb = None
        if has_rb or has_b2:
            ones_sb = const.tile([1, P], f32r, name="ones_sb")
            nc.vector.memset(ones_sb[:], 1.0)
        if has_rb:
            rb_sb = const.tile([1, E_ROUTED], f32, name="rb_sb")
            nc.sync.dma_start(rb_sb[:], rb_d[:])
        if has_b1:
            b1_sb = const.tile([P, I_TILES * E], f32, name="b1_sb")
            nc.sync.dma_start(b1_sb[:], b1_d[:])
        if has_b2:
            b2_sb = const.tile([1, E * D], f32r, name="b2_sb")
            nc.sync.dma_start(b2_sb[:], b2_d.rearrange("e d -> (e d)")[None, :])

        # ---- router: logits -> softmax -> top-2 gates -------------------
        gates = const.tile([P, T_TILES * E_ROUTED], f32, name="gates")
        psrs = [ps1.tile([P, FD], f32, name=f"psr{m}", tag="ps_h") for m in range(4)]
        psrs += [ps2.tile([P, FD], f32, name=f"psr{m+4}", tag="ps_y") for m in range(4)]
        GROUPS = [("h", "h"), ("h", "m"), ("h", "l"), ("m", "h"), ("m", "m"), ("l", "h")]
        NG = len(GROUPS)
        gi = 0
        cur_x = None
        for xp, rp in GROUPS:
            if xp != cur_x:
                # reloads ride SWDGE: their WAR waits would stall the SP
                # sequencer FIFO and everything queued behind it (xT, acc,
                # expert-0 weights); the Pool sequencer is otherwise idle
                for k in range(D_TILES):
                    nc.gpsimd.dma_start(
                        xs_sb[:, k * TOK : (k + 1) * TOK],
                        xs_d[xp][k * P : (k + 1) * P, :],
                    )
                cur_x = xp
            xv = xs_sb
            for m in range(T_TILES):
                pr = psrs[m][:, :E_ROUTED]
                for k in range(D_TILES):
                    nc.tensor.matmul(
                        pr,
                        lhsT=xv[:, k * TOK + m * P : k * TOK + (m + 1) * P],
                        rhs=rs_sb[rp][:, k * E_ROUTED : (k + 1) * E_ROUTED],
                        start=(gi == 0 and k == 0),
                        stop=(
                            gi == NG - 1 and k == D_TILES - 1 and not has_rb
                        ),
                    )
            gi += 1
        for m in range(T_TILES):
            pr = psrs[m][:, :E_ROUTED]
            if has_rb:
                nc.tensor.matmul(
                    pr, lhsT=ones_sb[:].bitcast(f32), rhs=rb_sb[:],
                    start=False, stop=True,
                )
            negmax = rtmp.tile([P, 1], f32, name="negmax")
            nc.vector.tensor_reduce(negmax, pr, axis=AX, op=OP.max, negate=True)
            ex = rtmp.tile([P, E_ROUTED], f32, name="ex")
            nc.scalar.activation(ex, pr, ACT.Exp, bias=negmax)
            ssum = rtmp.tile([P, 1], f32, name="ssum")
            nc.vector.reduce_sum(ssum, ex, axis=AX)
            rsum = rtmp.tile([P, 1], f32, name="rsum")
            nc.vector.reciprocal(rsum, ssum)
            aff = rtmp.tile([P, E_ROUTED], f32, name="aff")
            nc.vector.tensor_scalar_mul(aff, ex, rsum)
            m1 = rtmp.tile([P, 1], f32, name="m1")
            nc.vector.tensor_reduce(m1, aff, axis=AX, op=OP.max)
            is1 = rtmp.tile([P, E_ROUTED], f32, name="is1")
            nc.vector.tensor_scalar(is1, aff, m1, None, op0=OP.is_ge)
            aff2 = rtmp.tile([P, E_ROUTED], f32, name="aff2")
            nc.vector.scalar_tensor_tensor(
                aff2, in0=is1, scalar=-2.0, in1=aff, op0=OP.mult, op1=OP.add
            )
            m2 = rtmp.tile([P, 1], f32, name="m2")
            nc.vector.tensor_reduce(m2, aff2, axis=AX, op=OP.max)
            msk = rtmp.tile([P, E_ROUTED], f32, name="msk")
            nc.vector.tensor_scalar(msk, aff, m2, None, op0=OP.is_ge)
            nc.vector.tensor_mul(
                gates[:, m * E_ROUTED : (m + 1) * E_ROUTED], aff, msk
            )

        # xT / residual loads issued after the router so the startup
        # critical path is just xs_h + rs_h (~2MB) instead of ~10MB
        for k in range(D_TILES):
            nc.sync.dma_start(
                xT_sb[:, k * TOK : (k + 1) * TOK], xT_d[k * P : (k + 1) * P, :]
            )
        for m in range(T_TILES):
            nc.sync.dma_start(acc[m][:], x_d[m * P : (m + 1) * P, :])

        # ---- expert FFNs ------------------------------------------------
        for e in range(E):
            routed = e < E_ROUTED
            for k in range(D_TILES):
                nc.sync.dma_start(w1_sb[:, k * I : (k + 1) * I], w1_d[e, k, :, :])
            for k in range(I_TILES):
                nc.sync.dma_start(w2_sb[:, k * D : (k + 1) * D], w2_d[e, k, :, :])

            # mm1: h[it] = gelu(w1.T @ xT + b1), feature-major [I, tok]
            for it in range(I_TILES):
                pss = [
                    ps1.tile([P, FD], f32, name="ps_h", tag="ps_h")
                    for _ in range(NCH_TOK)
                ]
                for k in range(D_TILES):
                    for ch in range(NCH_TOK):
                        nc.tensor.matmul(
                            pss[ch],
                            lhsT=w1_sb[:, k * I + it * P : k * I + (it + 1) * P],
                            rhs=xT_sb[:, k * TOK + ch * FD : k * TOK + (ch + 1) * FD],
                            start=(k == 0),
                            stop=(k == D_TILES - 1),
                        )
                for ch in range(NCH_TOK):
                    bias = b1_sb[:, it * E + e : it * E + e + 1] if has_b1 else 0.0
                    nc.scalar.activation(
                        h_sb[:, it * TOK + ch * FD : it * TOK + (ch + 1) * FD],
                        pss[ch],
                        ACT.Gelu,
                        bias=bias,
                    )

            # mm2: token-major [tok, D]; then acc += gate * y (fused on DVE)
            for m in range(T_TILES):
                pss = [
                    ps2.tile([P, FD], f32, name="ps_y", tag="ps_y")
                    for _ in range(NCH_D)
                ]
                for k in range(I_TILES):
                    for ch in range(NCH_D):
                        nc.tensor.matmul(
                            pss[ch],
                            lhsT=h_sb[:, k * TOK + m * P : k * TOK + (m + 1) * P],
                            rhs=w2_sb[:, k * D + ch * FD : k * D + (ch + 1) * FD],
                            start=(k == 0),
                            stop=(k == I_TILES - 1 and not has_b2),
                        )
                if has_b2:
                    for ch in range(NCH_D):
                        nc.tensor.matmul(
                            pss[ch],
                            lhsT=ones_sb[:],
                            rhs=b2_sb[:, e * D + ch * FD : e * D + (ch + 1) * FD],
                            start=False,
                            stop=True,
                        )
                for ch in range(NCH_D):
                    a = acc[m][:, ch * FD : (ch + 1) * FD]
                    if routed:
                        g = gates[:, m * E_ROUTED + e : m * E_ROUTED + e + 1]
                        nc.vector.scalar_tensor_tensor(
                            a, in0=pss[ch], scalar=g, in1=a, op0=OP.mult, op1=OP.add
                        )
                    else:
                        nc.vector.tensor_add(a, a, pss[ch])

        # ---- store ------------------------------------------------------
        for m in range(T_TILES):
            nc.sync.dma_start(out_d[m * P : (m + 1) * P, :], acc[m][:])

    return nc


_CACHE: dict[tuple, bass.Bass] = {}


def _get_program(has_rb: bool, has_b1: bool, has_b2: bool) -> bass.Bass:
    key = (has_rb, has_b1, has_b2)
    if key not in _CACHE:
        nc = _build_program(*key)
        legal = _legalize_bir(nc.to_json_bytes())
        nc.to_json_bytes = lambda: legal  # shadow with the legalized BIR
        _CACHE[key] = nc
    return _CACHE[key]


def _prep_in_maps(inputs: dict[str, np.ndarray]):
    import ml_dtypes

    def split3(a):
        hi = a.astype(ml_dtypes.bfloat16)
        r = a - hi.astype(np.float32)
        mid = r.astype(ml_dtypes.bfloat16)
        lo = (r - mid.astype(np.float32)).astype(ml_dtypes.bfloat16)
        return {"h": hi, "m": mid, "l": lo}

    x = np.ascontiguousarray(np.asarray(inputs["x"], dtype=np.float32)).reshape(
        NTOK, D
    )
    rw = np.ascontiguousarray(np.asarray(inputs["router_w"], dtype=np.float32))
    rw_split = split3(rw)
    rb = np.asarray(inputs["router_b"], dtype=np.float32).reshape(1, E_ROUTED)
    w1_all = np.concatenate(
        [np.asarray(inputs["routed_w1"]), np.asarray(inputs["shared_w1"])]
    ).astype(np.float32)  # [E, D, I]
    b1_all = np.concatenate(
        [np.asarray(inputs["routed_b1"]), np.asarray(inputs["shared_b1"])]
    ).astype(np.float32)  # [E, I]
    w2_all = np.concatenate(
        [np.asarray(inputs["routed_w2"]), np.asarray(inputs["shared_w2"])]
    ).astype(np.float32)  # [E, I, D]
    b2_all = np.concatenate(
        [np.asarray(inputs["routed_b2"]), np.asarray(inputs["shared_b2"])]
    ).astype(np.float32)  # [E, D]

    has_rb = bool(np.any(rb))
    has_b1 = bool(np.any(b1_all))
    has_b2 = bool(np.any(b2_all))

    w1_r = np.ascontiguousarray(w1_all.reshape(E, D_TILES, P, I))
    w2_r = np.ascontiguousarray(w2_all.reshape(E, I_TILES, P, D))
    # b1 layout: [P, I_TILES*E] with column it*E+e = b1_all[e, it*P:(it+1)*P]
    b1_r = np.ascontiguousarray(
        b1_all.T.reshape(I_TILES, P, E).transpose(1, 0, 2).reshape(P, I_TILES * E)
    )

    in_maps = []
    for c in range(NCORES):
        xc = np.ascontiguousarray(x[c * TOK : (c + 1) * TOK, :])
        xTc = np.ascontiguousarray(xc.T)
        xT_split = split3(xTc)
        in_maps.append(
            {
                "xT": xTc,
                "x": xc,
                **{f"xs_{p}": np.ascontiguousarray(xT_split[p]) for p in "hml"},
                **{f"rs_{p}": np.ascontiguousarray(rw_split[p]) for p in "hml"},
                "rb": rb,
                "w1": w1_r,
                "b1": b1_r,
                "w2": w2_r,
                "b2": b2_all,
            }
        )
    return in_maps, (has_rb, has_b1, has_b2)


def kernel(**inputs: np.ndarray) -> np.ndarray:
    in_maps, flags = _prep_in_maps(inputs)
    nc = _get_program(*flags)
    res = run_bass_kernel_spmd(nc, in_maps, core_ids=list(range(NCORES)))
    outs = [res.results[c]["out"] for c in range(NCORES)]
    return np.concatenate(outs, axis=0).reshape(B, T, D).astype(np.float32)


# revision 55
# speedup vs baseline: 1.0032x; 1.0032x over previous
"""MoE layer (14 routed top-2 + 2 shared experts) on 8 trn2 NeuronCores.

Strategy: data-parallel over tokens. Each core takes 1024 of the 8192
tokens and runs the complete MoE for them: router matmul -> softmax ->
top-2 gate extraction, then all 16 expert FFNs computed densely with the
gate (zero for non-selected experts) folded into the accumulation. No
collectives needed; the host concatenates the 8 token shards.

Layouts (all chosen so no on-device transpose is ever needed):
  xT   [D, tok]   feature-major   -> mm1 moving operand + router lhsT
  h    [I, tok]   feature-major   -> mm1 output, mm2 stationary operand
  y    [tok, D]   token-major     -> mm2 output; gate is a per-partition
                                     scalar there, so gating+accumulate is
                                     one fused DVE scalar_tensor_tensor.
Matmuls run in float32r (full PE rate at moving-dim >= 256; fp32 bits).

Walrus codegen only accepts ONE sync-wait on most ISA structs (LDWEIGHTS,
TensorReduce, DMA, NoOp, Branch...), but Tile's scheduler freely emits
several per instruction. _legalize_bir() post-processes the serialized
BIR: same-semaphore waits are merged (max value) and every excess wait is
hoisted onto an injected single-wait NoOp right before the instruction on
the same engine queue — semantically identical, structurally legal.
"""

import os
import sys

sys.path.insert(0, "/opt/trn_rl_repo")
# recover cleanly if a previous run left the NeuronCores wedged
os.environ.setdefault("NEURON_RT_RESET_CORES", "1")

from contextlib import ExitStack

import numpy as np
import orjson

import concourse.bass as bass
import concourse.tile as tile
from concourse import mybir
from concourse.bass_utils import run_bass_kernel_spmd

P = 128
B, T, D = 4, 2048, 1024
I = 1024
E_ROUTED = 14
N_SHARED = 2
E = E_ROUTED + N_SHARED  # 16 FFNs total; 0..13 routed (gated), 14..15 shared
NCORES = 8
NTOK = B * T  # 8192
TOK = NTOK // NCORES  # 1024 tokens per core
T_TILES = TOK // P  # 8
D_TILES = D // P  # 8
I_TILES = I // P  # 8
FD = 512  # matmul moving free dim (fp32 max)
NCH_TOK = TOK // FD  # 2
NCH_D = D // FD  # 2

f32 = mybir.dt.float32
f32r = mybir.dt.float32r
AX = mybir.AxisListType.X
OP = mybir.AluOpType
ACT = mybir.ActivationFunctionType


def _legalize_bir(js_bytes: bytes) -> bytes:
    """Merge same-sem waits and hoist excess waits to injected NoOps so no
    instruction carries more than one sync-wait (walrus struct limits)."""
    js = orjson.loads(js_bytes)
    n = 0
    for fn in js["functions"]:
        for blk in fn["blocks"]:
            new_insts = []
            for inst in blk["instructions"]:
                si = inst.get("sync_info")
                waits = (si or {}).get("on_wait") or []
                if len(waits) > 1:
                    merged, order, other = {}, [], []
                    for w in waits:
                        if (
                            w.get("wait_mode") == "sem-ge-imm"
                            and w.get("sync_type") == "semaphore"
                            and w.get("wait_reg") is None
                        ):
                            k = w["id"]
                            if k in merged:
                                if w["wait_value"] > merged[k]["wait_value"]:
                                    merged[k] = w
                            else:
                                merged[k] = w
                                order.append(k)
                        else:
                            other.append(w)
                    ws = [merged[k] for k in order] + other
                    for w in ws[:-1]:
                        n += 1
                        new_insts.append(
                            {
                                "debug": inst.get("debug", 0),
                                "engine": inst["engine"],
                                "name": f"I-lgl-{n}",
                                "opcode": "NoOp",
                                "sync_info": {"on_update": [], "on_wait": [w]},
                            }
                        )
                    si["on_wait"] = ws[-1:]
                new_insts.append(inst)
            blk["instructions"] = new_insts
    return orjson.dumps(js)


def _build_program(has_rb: bool, has_b1: bool, has_b2: bool) -> bass.Bass:
    nc = bass.Bass()

    xT_d = nc.dram_tensor("xT", [D, TOK], f32r, kind="ExternalInput")
    x_d = nc.dram_tensor("x", [TOK, D], f32, kind="ExternalInput")
    # router inputs as 3-way bf16 splits (x = xh+xm+xl exactly, same for
    # rw): 6 bf16 product groups accumulated in fp32 PSUM give true-fp32
    # logits; the PE's native fp32 mode is ~1e-5 off, enough to flip top-2
    # selections on tokens whose #2/#3 affinity gap is ~4e-6.
    bf16 = mybir.dt.bfloat16
    xs_d = {
        p: nc.dram_tensor(f"xs_{p}", [D, TOK], bf16, kind="ExternalInput")
        for p in "hml"
    }
    rs_d = {
        p: nc.dram_tensor(f"rs_{p}", [D, E_ROUTED], bf16, kind="ExternalInput")
        for p in "hml"
    }
    rb_d = nc.dram_tensor("rb", [1, E_ROUTED], f32, kind="ExternalInput")
    w1_d = nc.dram_tensor("w1", [E, D_TILES, P, I], f32r, kind="ExternalInput")
    b1_d = nc.dram_tensor("b1", [P, I_TILES * E], f32, kind="ExternalInput")
    w2_d = nc.dram_tensor("w2", [E, I_TILES, P, D], f32r, kind="ExternalInput")
    b2_d = nc.dram_tensor("b2", [E, D], f32r, kind="ExternalInput")
    out_d = nc.dram_tensor("out", [TOK, D], f32, kind="ExternalOutput")

    with tile.TileContext(nc) as tc, ExitStack() as ctx:
        const = ctx.enter_context(tc.tile_pool(name="const", bufs=1))
        rtmp = ctx.enter_context(tc.tile_pool(name="rtmp", bufs=T_TILES))
        ps1 = ctx.enter_context(tc.tile_pool(name="ps1", bufs=4, space="PSUM"))
        ps2 = ctx.enter_context(tc.tile_pool(name="ps2", bufs=4, space="PSUM"))

        # ---- resident tensors -------------------------------------------
        xT_sb = const.tile([P, D_TILES * TOK], f32r, name="xT_sb")
        xs_sb = const.tile([P, D_TILES * TOK], mybir.dt.bfloat16, name="xs_sb")
        rs_sb = {}
        for p in "hml":
            t = const.tile(
                [P, D_TILES * E_ROUTED], mybir.dt.bfloat16, name=f"rs_{p}"
            )
            for k in range(D_TILES):
                nc.sync.dma_start(
                    t[:, k * E_ROUTED : (k + 1) * E_ROUTED],
                    rs_d[p][k * P : (k + 1) * P, :],
                )
            rs_sb[p] = t
        acc = [const.tile([P, D], f32, name=f"acc{m}") for m in range(T_TILES)]
        h_sb = const.tile([P, I_TILES * TOK], f32r, name="h_sb")
        # Persistent weight tiles, allocated once: per-expert overwrites are
        # tracked at range level (pool slot realloc would pull every previous
        # accessor into the dep set).
        w1_sb = const.tile([P, D_TILES * I], f32r, name="w1_sb")
        w2_sb = const.tile([P, I_TILES * D], f32r, name="w2_sb")

        ones_sb = None
        if has_rb or has_b2:
            ones_sb = const.tile([1, P], f32r, name="ones_sb")
            nc.vector.memset(ones_sb[:], 1.0)
        if has_rb:
            rb_sb = const.tile([1, E_ROUTED], f32, name="rb_sb")
            nc.sync.dma_start(rb_sb[:], rb_d[:])
        if has_b1:
            b1_sb = const.tile([P, I_TILES * E], f32, name="b1_sb")
            nc.sync.dma_start(b1_sb[:], b1_d[:])
        if has_b2:
            b2_sb = const.tile([1, E * D], f32r, name="b2_sb")
            nc.sync.dma_start(b2_sb[:], b2_d.rearrange("e d -> (e d)")[None, :])

        # ---- router: logits -> softmax -> top-2 gates -------------------
        gates = const.tile([P, T_TILES * E_ROUTED], f32, name="gates")
        psrs = [ps1.tile([P, FD], f32, name=f"psr{m}", tag="ps_h") for m in range(4)]
        psrs += [ps2.tile([P, FD], f32, name=f"psr{m+4}", tag="ps_y") for m in range(4)]
        GROUPS = [("h", "h"), ("h", "m"), ("h", "l"), ("m", "h"), ("m", "m"), ("l", "h")]
        NG = len(GROUPS)
        gi = 0
        cur_x = None
        for xp, rp in GROUPS:
            if xp != cur_x:
                for k in range(D_TILES):
                    nc.sync.dma_start(
                        xs_sb[:, k * TOK : (k + 1) * TOK],
                        xs_d[xp][k * P : (k + 1) * P, :],
                    )
                cur_x = xp
            xv = xs_sb
            for m in range(T_TILES):
                pr = psrs[m][:, :E_ROUTED]
                for k in range(D_TILES):
                    nc.tensor.matmul(
                        pr,
                        lhsT=xv[:, k * TOK + m * P : k * TOK + (m + 1) * P],
                        rhs=rs_sb[rp][:, k * E_ROUTED : (k + 1) * E_ROUTED],
                        start=(gi == 0 and k == 0),
                        stop=(
                            gi == NG - 1 and k == D_TILES - 1 and not has_rb
                        ),
                    )
            gi += 1
        for m in range(T_TILES):
            pr = psrs[m][:, :E_ROUTED]
            if has_rb:
                nc.tensor.matmul(
                    pr, lhsT=ones_sb[:].bitcast(f32), rhs=rb_sb[:],
                    start=False, stop=True,
                )
            negmax = rtmp.tile([P, 1], f32, name="negmax")
            nc.vector.tensor_reduce(negmax, pr, axis=AX, op=OP.max, negate=True)
            ex = rtmp.tile([P, E_ROUTED], f32, name="ex")
            nc.scalar.activation(ex, pr, ACT.Exp, bias=negmax)
            ssum = rtmp.tile([P, 1], f32, name="ssum")
            nc.vector.reduce_sum(ssum, ex, axis=AX)
            rsum = rtmp.tile([P, 1], f32, name="rsum")
            nc.vector.reciprocal(rsum, ssum)
            aff = rtmp.tile([P, E_ROUTED], f32, name="aff")
            nc.vector.tensor_scalar_mul(aff, ex, rsum)
            m1 = rtmp.tile([P, 1], f32, name="m1")
            nc.vector.tensor_reduce(m1, aff, axis=AX, op=OP.max)
            is1 = rtmp.tile([P, E_ROUTED], f32, name="is1")
            nc.vector.tensor_scalar(is1, aff, m1, None, op0=OP.is_ge)
            aff2 = rtmp.tile([P, E_ROUTED], f32, name="aff2")
            nc.vector.scalar_tensor_tensor(
                aff2, in0=is1, scalar=-2.0, in1=aff, op0=OP.mult, op1=OP.add
            )
            m2 = rtmp.tile([P, 1], f32, name="m2")
            nc.vector.tensor_reduce(m2, aff2, axis=AX, op=OP.max)
            msk = rtmp.tile([P, E_ROUTED], f32, name="msk")
            nc.vector.tensor_scalar(msk, aff, m2, None, op0=OP.is_ge)
            nc.vector.tensor_mul(
                gates[:, m * E_ROUTED : (m + 1) * E_ROUTED], aff, msk
            )

        # xT / residual loads issued after the router so the startup
        # critical path is just xs_h + rs_h (~2MB) instead of ~10MB
        for k in range(D_TILES):
            nc.sync.dma_start(
                xT_sb[:, k * TOK : (k + 1) * TOK], xT_d[k * P : (k + 1) * P, :]
            )
        for m in range(T_TILES):
            nc.sync.dma_start(acc[m][:], x_d[m * P : (m + 1) * P, :])

        # ---- expert FFNs ------------------------------------------------
        for e in range(E):
            routed = e < E_ROUTED
            for k in range(D_TILES):
                nc.sync.dma_start(w1_sb[:, k * I : (k + 1) * I], w1_d[e, k, :, :])
            for k in range(I_TILES):
                nc.sync.dma_start(w2_sb[:, k * D : (k + 1) * D], w2_d[e, k, :, :])

            # mm1: h[it] = gelu(w1.T @ xT + b1), feature-major [I, tok]
            for it in range(I_TILES):
                pss = [
                    ps1.tile([P, FD], f32, name="ps_h", tag="ps_h")
                    for _ in range(NCH_TOK)
                ]
                for k in range(D_TILES):
                    for ch in range(NCH_TOK):
                        nc.tensor.matmul(
                            pss[ch],
                            lhsT=w1_sb[:, k * I + it * P : k * I + (it + 1) * P],
                            rhs=xT_sb[:, k * TOK + ch * FD : k * TOK + (ch + 1) * FD],
                            start=(k == 0),
                            stop=(k == D_TILES - 1),
                        )
                for ch in range(NCH_TOK):
                    bias = b1_sb[:, it * E + e : it * E + e + 1] if has_b1 else 0.0
                    nc.scalar.activation(
                        h_sb[:, it * TOK + ch * FD : it * TOK + (ch + 1) * FD],
                        pss[ch],
                        ACT.Gelu,
                        bias=bias,
                    )

            # mm2: token-major [tok, D]; then acc += gate * y (fused on DVE)
            for m in range(T_TILES):
                pss = [
                    ps2.tile([P, FD], f32, name="ps_y", tag="ps_y")
                    for _ in range(NCH_D)
                ]
                for k in range(I_TILES):
                    for ch in range(NCH_D):
                        nc.tensor.matmul(
                            pss[ch],
                            lhsT=h_sb[:, k * TOK + m * P : k * TOK + (m + 1) * P],
                            rhs=w2_sb[:, k * D + ch * FD : k * D + (ch + 1) * FD],
                            start=(k == 0),
                            stop=(k == I_TILES - 1 and not has_b2),
                        )
                if has_b2:
                    for ch in range(NCH_D):
                        nc.tensor.matmul(
                            pss[ch],
                            lhsT=ones_sb[:],
                            rhs=b2_sb[:, e * D + ch * FD : e * D + (ch + 1) * FD],
                            start=False,
                            stop=True,
                        )
                for ch in range(NCH_D):
                    a = acc[m][:, ch * FD : (ch + 1) * FD]
                    if routed:
                        g = gates[:, m * E_ROUTED + e : m * E_ROUTED + e + 1]
                        nc.vector.scalar_tensor_tensor(
                            a, in0=pss[ch], scalar=g, in1=a, op0=OP.mult, op1=OP.add
                        )
                    else:
                        nc.vector.tensor_add(a, a, pss[ch])

        # ---- store ------------------------------------------------------
        for m in range(T_TILES):
            nc.sync.dma_start(out_d[m * P : (m + 1) * P, :], acc[m][:])

    return nc


_CACHE: dict[tuple, bass.Bass] = {}


def _get_program(has_rb: bool, has_b1: bool, has_b2: bool) -> bass.Bass:
    key = (has_rb, has_b1, has_b2)
    if key not in _CACHE:
        nc = _build_program(*key)
        legal = _legalize_bir(nc.to_json_bytes())
        nc.to_json_bytes = lambda: legal  # shadow with the legalized BIR
        _CACHE[key] = nc
    return _CACHE[key]


def _prep_in_maps(inputs: dict[str, np.ndarray]):
    import ml_dtypes

    def split3(a):
        hi = a.astype(ml_dtypes.bfloat16)
        r = a - hi.astype(np.float32)
        mid = r.astype(ml_dtypes.bfloat16)
        lo = (r - mid.astype(np.float32)).astype(ml_dtypes.bfloat16)
        return {"h": hi, "m": mid, "l": lo}

    x = np.ascontiguousarray(np.asarray(inputs["x"], dtype=np.float32)).reshape(
        NTOK, D
    )
    rw = np.ascontiguousarray(np.asarray(inputs["router_w"], dtype=np.float32))
    rw_split = split3(rw)
    rb = np.asarray(inputs["router_b"], dtype=np.float32).reshape(1, E_ROUTED)
    w1_all = np.concatenate(
        [np.asarray(inputs["routed_w1"]), np.asarray(inputs["shared_w1"])]
    ).astype(np.float32)  # [E, D, I]
    b1_all = np.concatenate(
        [np.asarray(inputs["routed_b1"]), np.asarray(inputs["shared_b1"])]
    ).astype(np.float32)  # [E, I]
    w2_all = np.concatenate(
        [np.asarray(inputs["routed_w2"]), np.asarray(inputs["shared_w2"])]
    ).astype(np.float32)  # [E, I, D]
    b2_all = np.concatenate(
        [np.asarray(inputs["routed_b2"]), np.asarray(inputs["shared_b2"])]
    ).astype(np.float32)  # [E, D]

    has_rb = bool(np.any(rb))
    has_b1 = bool(np.any(b1_all))
    has_b2 = bool(np.any(b2_all))

    w1_r = np.ascontiguousarray(w1_all.reshape(E, D_TILES, P, I))
    w2_r = np.ascontiguousarray(w2_all.reshape(E, I_TILES, P, D))
    # b1 layout: [P, I_TILES*E] with column it*E+e = b1_all[e, it*P:(it+1)*P]
    b1_r = np.ascontiguousarray(
        b1_all.T.reshape(I_TILES, P, E).transpose(1, 0, 2).reshape(P, I_TILES * E)
    )

    in_maps = []
    for c in range(NCORES):
        xc = np.ascontiguousarray(x[c * TOK : (c + 1) * TOK, :])
        xTc = np.ascontiguousarray(xc.T)
        xT_split = split3(xTc)
        in_maps.append(
            {
                "xT": xTc,
                "x": xc,
                **{f"xs_{p}": np.ascontiguousarray(xT_split[p]) for p in "hml"},
                **{f"rs_{p}": np.ascontiguousarray(rw_split[p]) for p in "hml"},
                "rb": rb,
                "w1": w1_r,
                "b1": b1_r,
                "w2": w2_r,
                "b2": b2_all,
            }
        )
    return in_maps, (has_rb, has_b1, has_b2)


def kernel(**inputs: np.ndarray) -> np.ndarray:
    in_maps, flags = _prep_in_maps(inputs)
    nc = _get_program(*flags)
    res = run_bass_kernel_spmd(nc, in_maps, core_ids=list(range(NCORES)))
    outs = [res.results[c]["out"] for c in range(NCORES)]
    return np.concatenate(outs, axis=0).reshape(B, T, D).astype(np.float32)
